# revision 9
# baseline (speedup 1.0000x reference)
"""Trainium2 Bass kernel for nn_Decoder sparse-attention decode step.

Reference computation (n=200000, d=128):
    f = concat([x, X[s], X[p]]); q = f @ Wq
    u = (X @ Wk) @ q / sqrt(d)
    u_ = softmax(u + mask)          # mask: 1 everywhere, 0 at visited
    out = (u_ @ (X @ Wv)) @ Wo

Algebraic restructure (exact in exact arithmetic):
    w   = Wk @ q / sqrt(d)                      # [d]
    u   = X @ w
    softmax(u + mask) = softmax(u - ind_visited)  (shift by -1)
    acc = sum_r exp(u_r) X_r ; S = sum_r exp(u_r)       (UNMASKED)
    accv = sum_visited exp(u_r) X_r ; Sv = sum_visited exp(u_r)
    out = Wvo^T (acc - (1-1/e) accv) / (S - npad - (1-1/e) Sv'),  Wvo = Wv Wo

Sharding: X rows split across 8 NeuronCores (25000 rows each, zero-padded to
25088 cols), stored FEATURE-MAJOR (Xf[f, r] = X[r, f], bf16).  Each core
outputs [Wvo^T acc | S | Wvo^T accv | Sv]; the host applies the linear
visited/pad corrections and divides.

Why feature-major: the cost model prices a matmul by its OUTPUT free size
only (stationary weights are free), so with partition=feature:
  - u-pass: matmul(lhsT=w broadcast, rhs=Xf chunk) -> u replicated in PSUM
    at ~1 PE cycle per row (bf16).
  - exp: ACT reads the PSUM chunk directly (no PSUM->SBUF copy), writes
    replicated p (bf16) + accumulates S.
  - acc-pass: one DVE scalar_tensor_tensor per chunk (Xf * p, accum -> acc
    column).  DVE is the only engine that can do this contraction; chunked
    at 3584 columns its fixed overhead is negligible, leaving the
    irreducible 1 elem/cycle/lane pass (~26us) as the kernel's bottleneck,
    vs ~38us for the row-major per-tile dot formulation (196 x 128-elem
    STTs whose per-instruction overhead can't be amortized).
Visited rows are host-gathered into a 384-column feature-major block and
corrected via the same pipeline (linearity of acc/S), off the critical path.

bf16 X halves the HBM stream (6.4MB/core, ~18.6us DMA) and stays well
inside the 2e-2 rel-err budget (measured ~5e-3).
"""

import os
import sys

import numpy as np

_REPO = "/opt/trn_rl_repo"
if _REPO not in sys.path:
    sys.path.insert(0, _REPO)

import ml_dtypes

import concourse.bacc as bacc
import concourse.bass_utils as bass_utils
import concourse.mybir as mybir
from concourse import tile

P = 128                    # hidden dim / partition count
NCORES = 8
NROWS = 25000              # rows per core
RP = 25088                 # padded rows per core (multiple of 512)
NPAD = RP - NROWS
VN = 384                   # visited-gather columns (max visited rows/core)
EINV = float(np.exp(-1.0))

# X column chunks. Must be multiples of 512 (PSUM bank = 512 f32; each
# matmul output may not cross a bank) and at most PSUM_W. Small leading
# chunks start the pipeline early; small trailing chunk shortens the tail.
_CH_ENV = os.environ.get("KCH", "")
if _CH_ENV:
    CH = [int(c) for c in _CH_ENV.split(",")]
else:
    CH = [512, 1024, 3584, 3584, 3584, 3584, 3584, 3584, 2048]
assert sum(CH) == RP and all(c % 512 == 0 for c in CH)
NCHUNK = len(CH)
CHOFF = [sum(CH[:c]) for c in range(NCHUNK)]
PSUM_W = max(CH)           # single rotating PSUM slot for u

F32 = mybir.dt.float32
BF16 = mybir.dt.bfloat16

# cpack (bf16) column layout
_C_WQ = 0                      # [0:384)      wqT as [c, j*128+m]
_C_WK = 384                    # [384:512)    wkT = Wk.T
_C_FV = 512                    # [512:515)    fvecT columns [x, X[s], X[p]]
_C_XV = 516                    # [516:900)    visited rows, feature-major
_C_END = 900

_CACHE = {}


def _build_program():
    if "nc" in _CACHE:
        return _CACHE["nc"]

    nc = bacc.Bacc(
        "TRN2",
        target_bir_lowering=False,
        debug=False,
        enable_asserts=False,
        num_devices=NCORES,
    )

    xs_d = nc.dram_tensor("xs", [P, RP], BF16, kind="ExternalInput")
    cp_d = nc.dram_tensor("cpack", [P, _C_END], BF16, kind="ExternalInput")
    wvo_d = nc.dram_tensor("wvo", [P, P], F32, kind="ExternalInput")
    # cols: 0 = Wvo^T acc, 1 = S (all partitions), 2 = Wvo^T accv, 3 = Sv
    o_d = nc.dram_tensor("o_part", [P, 4], F32, kind="ExternalOutput")

    with tile.TileContext(nc) as tc:
        with (
            tc.tile_pool(name="psum_u", bufs=1, space="PSUM") as pu,
            tc.tile_pool(name="psum_misc", bufs=1, space="PSUM") as pm,
            tc.tile_pool(name="const", bufs=1) as cpool,
            tc.tile_pool(name="xpool", bufs=1) as xpool,
            tc.tile_pool(name="work", bufs=1) as wpool,
            tc.tile_pool(name="scratch", bufs=2) as spool,
        ):
            # ---- constants: one packed DMA ----
            cp_sb = cpool.tile([P, _C_END], BF16, tag="cpack")
            nc.sync.dma_start(cp_sb[:], cp_d.ap())
            wq_sb = cp_sb[:, _C_WQ:_C_WK].rearrange("p (j f) -> p j f", j=3)
            wk_sb = cp_sb[:, _C_WK:_C_FV]
            fv_sb = cp_sb[:, _C_FV : _C_FV + 3]
            xv_sb = cp_sb[:, _C_XV:_C_END]
            wvo_sb = cpool.tile([P, P], F32, tag="wvo")
            nc.sync.dma_start(wvo_sb[:], wvo_d.ap())

            # ---- X chunks: all DMAs issued up front ----
            x_sb = []
            for c, w in enumerate(CH):
                xt = xpool.tile([P, w], BF16, tag=f"x{c}", name=f"x{c}")
                nc.sync.dma_start(xt[:], xs_d.ap()[:, CHOFF[c] : CHOFF[c] + w])
                x_sb.append(xt)

            # ---- prologue: q = f @ Wq ; w = Wk q / sqrt(d) ----
            # single PSUM bank shared by all small matmul outputs (each
            # accumulation group is opened and closed sequentially on PE)
            misc_ps = pm.tile([P, 512], F32, tag="misc")
            q_ps = misc_ps[:, 0:1]
            for j in range(3):
                nc.tensor.matmul(
                    q_ps,
                    wq_sb[:, j, :],
                    fv_sb[:, j : j + 1],
                    start=(j == 0),
                    stop=(j == 2),
                )
            q_sb = wpool.tile([P, 1], BF16, tag="q_sb")
            nc.scalar.copy(q_sb[:], q_ps[:])
            # wcol[f] = sum_c wkT[c, f] q[c]  (wkT = Wk.T so this is Wk @ q)
            wc_ps = misc_ps[:, 1:2]
            nc.tensor.matmul(wc_ps[:], wk_sb[:], q_sb[:])
            wc_sb = wpool.tile([P, 1], BF16, tag="wc_sb")
            nc.scalar.mul(wc_sb[:], wc_ps[:], 1.0 / float(np.sqrt(np.float32(P))))
            wb_sb = wc_sb[:].broadcast_to([P, P])  # lhsT: w on every column

            # ---- main streaming loop (single rotating PSUM slot) ----
            acc_sb = wpool.tile([P, NCHUNK], F32, tag="acc_cols")
            sc_sb = wpool.tile([P, NCHUNK], F32, tag="s_cols")
            for c, w in enumerate(CH):
                up = pu.tile([P, PSUM_W], F32, tag="ups", name=f"up{c}")
                for k in range(w // 512):
                    nc.tensor.matmul(
                        up[:, k * 512 : (k + 1) * 512],
                        wb_sb,
                        x_sb[c][:, k * 512 : (k + 1) * 512],
                        start=True,
                        stop=True,
                    )
                pt = wpool.tile([P, w], BF16, tag=f"p{c}", name=f"p{c}")
                nc.scalar.activation(
                    pt[:],
                    up[:, 0:w],
                    mybir.ActivationFunctionType.Exp,
                    accum_out=sc_sb[:, c : c + 1],
                )
                scr = spool.tile([P, PSUM_W], BF16, tag="scr", name="scr")
                nc.vector.scalar_tensor_tensor(
                    out=scr[:, 0:w],
                    in0=x_sb[c][:],
                    scalar=1.0,
                    in1=pt[:],
                    op0=mybir.AluOpType.mult,
                    op1=mybir.AluOpType.mult,
                    accum_out=acc_sb[:, c : c + 1],
                )

            # ---- visited correction (same pipeline, off critical path) ----
            uv_ps = misc_ps[:, 128 : 128 + VN]
            nc.tensor.matmul(uv_ps[:], wb_sb, xv_sb[:], start=True, stop=True)
            pv_sb = wpool.tile([P, VN], BF16, tag="pv")
            opk_sb = wpool.tile([P, 4], F32, tag="opk")
            nc.scalar.activation(
                pv_sb[:],
                uv_ps[:],
                mybir.ActivationFunctionType.Exp,
                accum_out=opk_sb[:, 3:4],
            )
            scrv = spool.tile([P, VN], BF16, tag="scrv", name="scrv")
            av_sb = wpool.tile([P, 1], F32, tag="av")
            nc.vector.scalar_tensor_tensor(
                out=scrv[:],
                in0=xv_sb[:],
                scalar=1.0,
                in1=pv_sb[:],
                op0=mybir.AluOpType.mult,
                op1=mybir.AluOpType.mult,
                accum_out=av_sb[:],
            )
            ov_ps = misc_ps[:, 3:4]
            nc.tensor.matmul(ov_ps[:], wvo_sb[:], av_sb[:])
            nc.scalar.copy(opk_sb[:, 2:3], ov_ps[:])

            # ---- epilogue ----
            nc.vector.tensor_reduce(
                opk_sb[:, 1:2], sc_sb[:], mybir.AxisListType.X, mybir.AluOpType.add
            )
            am_sb = wpool.tile([P, 1], F32, tag="am")
            nc.vector.tensor_reduce(
                am_sb[:], acc_sb[:], mybir.AxisListType.X, mybir.AluOpType.add
            )
            o_ps = misc_ps[:, 2:3]
            nc.tensor.matmul(o_ps[:], wvo_sb[:], am_sb[:])
            nc.scalar.copy(opk_sb[:, 0:1], o_ps[:])
            nc.sync.dma_start(o_d.ap(), opk_sb[:])

    nc.compile()
    _CACHE["nc"] = nc
    return nc


def make_in_maps(X, x, Wq, Wk, Wv, Wo, nodes_visited, starting_node, previous_node):
    X = np.asarray(X, dtype=np.float32)
    x = np.asarray(x, dtype=np.float32)
    Wq = np.asarray(Wq, dtype=np.float32)
    Wk = np.asarray(Wk, dtype=np.float32)
    Wv = np.asarray(Wv, dtype=np.float32)
    Wo = np.asarray(Wo, dtype=np.float32)
    vis = np.unique(np.asarray(nodes_visited).astype(np.int64))

    fvecT = np.stack([x, X[int(starting_node)], X[int(previous_node)]], axis=1)
    wqT = Wq.reshape(3, P, P).transpose(1, 0, 2).reshape(P, 3 * P)
    wkT = Wk.T
    wvo = (Wv.astype(np.float64) @ Wo.astype(np.float64)).astype(np.float32)

    in_maps = []
    vispad = []
    for c in range(NCORES):
        lo, hi = c * NROWS, (c + 1) * NROWS
        xs = np.zeros((RP, P), np.float32)
        xs[:NROWS] = X[lo:hi]
        sel = vis[(vis >= lo) & (vis < hi)] - lo
        n = len(sel)
        assert n <= VN, f"core {c}: {n} visited rows exceed VN={VN}"
        xv = np.zeros((VN, P), np.float32)
        xv[:n] = X[lo:hi][sel]
        vispad.append(VN - n)
        cpack = np.zeros((P, _C_END), ml_dtypes.bfloat16)
        cpack[:, _C_WQ:_C_WK] = wqT.astype(ml_dtypes.bfloat16)
        cpack[:, _C_WK:_C_FV] = wkT.astype(ml_dtypes.bfloat16)
        cpack[:, _C_FV : _C_FV + 3] = fvecT.astype(ml_dtypes.bfloat16)
        cpack[:, _C_XV:_C_END] = xv.T.astype(ml_dtypes.bfloat16)
        in_maps.append(
            {
                "xs": np.ascontiguousarray(xs.T).astype(ml_dtypes.bfloat16),
                "cpack": cpack,
                "wvo": wvo,
            }
        )
    make_in_maps.vispad = vispad
    return in_maps


def combine(results, vispad):
    c1 = 1.0 - EINV
    o = np.zeros(P, np.float64)
    S = 0.0
    for r, vp in zip(results, vispad):
        op = r["o_part"].astype(np.float64)
        o += op[:, 0] - c1 * op[:, 2]
        S += (op[0, 1] - NPAD) - c1 * (op[0, 3] - vp)
    return (o / S).astype(np.float32)


def kernel(X, x, Wq, Wk, Wv, Wo, nodes_visited, starting_node, previous_node,
           _trace=False):
    nc = _build_program()
    in_maps = make_in_maps(
        X, x, Wq, Wk, Wv, Wo, nodes_visited, starting_node, previous_node
    )
    res = bass_utils.run_bass_kernel_spmd(
        nc, in_maps, core_ids=list(range(NCORES)), trace=_trace
    )
    out = combine(res.results, make_in_maps.vispad)
    if _trace:
        kernel.last_exec_time_ns = res.exec_time_ns
        kernel.last_profile = res.profile_json
    return out
